# revision 12
# baseline (speedup 1.0000x reference)
"""MoE layer (hash-routed, top-k=2, E=8 experts) on 8 Trainium2 NeuronCores.

Strategy: expert-parallel. Core e holds expert e's weights (W1[e], W2[e]).
The host routes: for each expert, gather the distinct tokens assigned to it
(assign[b,s,:] contains expert ids; a token contributes once per distinct
expert), transpose the gathered activations to [D, C] so the device never
has to transpose, run a dense 2-layer MLP per core, then scatter-add the
per-expert outputs back and divide by k.

Device kernel (per core), C = token capacity (padded, multiple of 128),
two phases, matmul operands in bf16 (fp32 PSUM accumulation):

  phase 1: for each of 32 h-tiles:
      H1T[h,tok] = relu(W1[ht]^T @ XT + b1)   (PSUM-accum over 8 d-tiles;
                                               each stationary w1 tile
                                               streams all C tokens, so one
                                               LDWEIGHTS serves 2 matmuls)
      full H1T [4096 x C] stays in SBUF (bf16)
  phase 2: for each 128-token tile:
      Y[tok,d] = H1T^T @ W2                   (PSUM pair accumulates over
                                               all 32 h-tiles, then copy
                                               to SBUF and DMA out)

All DMA rides the sync HWDGE ring (the SWDGE path costs ~2us per op and
fine-grained patterns make it far slower still).  Host pre-lays-out b1 as
[128, HT] and W1 as [128, HT, KT*128] so every transfer moves >=1KB
contiguous runs per partition (descriptor-lean; the original transposed
b1 pattern emitted 4096 4-byte descriptors and alone stalled the ring
for ~23us).  Emission follows exact consumption order: b1, w1[0], xt
(kt-major), w1[1], then per h-tile {w1 two ahead, w2 piece}, then y
stores.  A short PE warm-up bridges the ~7us NEFF preamble until the
first pieces land so real matmuls start at full clock (HAM 8/8).
"""

import math
import numpy as np
import ml_dtypes

import concourse.bass as bass
import concourse.mybir as mybir
import concourse.tile as tile
from concourse import bacc
from concourse.bass_utils import run_bass_kernel_spmd

dt = mybir.dt

B, S, D, H, E, NCORES = 4, 1024, 1024, 4096, 8, 8
KT = D // 128                  # 8 contraction tiles (d)
HT = H // 128                  # 32 h tiles
BF16 = np.dtype(ml_dtypes.bfloat16)

_BUILD_CACHE: dict = {}


def build_nc(C: int):
    """Build + compile the per-core Bass program for token capacity C."""
    assert C % 128 == 0
    TT = C // 128
    io_dt = dt.bfloat16

    nc = bacc.Bacc(
        "TRN2",
        target_bir_lowering=False,
        debug=False,
        num_devices=NCORES,
    )

    # host-side layouts (see kernel()): w1 [128, HT, KT*128] with
    # w1[p, ht, kt*128+j] = W1[kt*128+p, ht*128+j]; b1 [128, HT] with
    # b1[p, ht] = b1_vec[ht*128+p]
    xt_d = nc.dram_tensor("xt", [D, C], io_dt, kind="ExternalInput")
    w1_d = nc.dram_tensor("w1", [128, HT, KT * 128], io_dt, kind="ExternalInput")
    b1_d = nc.dram_tensor("b1", [128, HT], dt.float32, kind="ExternalInput")
    w2_d = nc.dram_tensor("w2", [H, D], io_dt, kind="ExternalInput")
    y_d = nc.dram_tensor("y", [C, D], dt.float32, kind="ExternalOutput")

    xt_v = xt_d.ap().rearrange("(kt p) c -> p kt c", p=128)
    w2_v = w2_d.ap().rearrange("(hh p) d -> p hh d", p=128)
    y_v = y_d.ap().rearrange("(tt p) d -> p tt d", p=128)

    chunks = [(c0, min(512, C - c0)) for c0 in range(0, C, 512)]

    # SBUF per-partition budget (bytes)
    need = (
        KT * C * 2            # xt
        + HT * C * 2          # h1
        + HT * 1024 * 2       # w2 (all of it)
        + 3 * KT * 128 * 2    # w1 stream
        + 3 * 512 * 4         # y stage
        + HT * 4              # b1
        + 512 * 2             # warm tile
    )
    assert need <= 190 * 1024, f"SBUF over budget: {need // 1024}KB for C={C}"

    with tile.TileContext(nc) as tc:
        with (
            tc.tile_pool(name="xt", bufs=1) as xt_pool,
            tc.tile_pool(name="b1", bufs=1) as b1_pool,
            tc.tile_pool(name="w1s", bufs=3) as w1_pool,
            tc.tile_pool(name="w2a", bufs=1) as w2_pool,
            tc.tile_pool(name="h1", bufs=1) as h1_pool,
            tc.tile_pool(name="yst", bufs=3) as yst_pool,
            tc.tile_pool(name="warm", bufs=1) as warm_pool,
            tc.tile_pool(name="ps1", bufs=4, space="PSUM") as ps1_pool,
            tc.tile_pool(name="ps2", bufs=4, space="PSUM") as ps2_pool,
        ):
            # PE warm-up: dependency-free bf16 matmuls bridge the NEFF
            # preamble until the first xt/w1 pieces land (~10us), so the
            # HAM clock gate reaches 8/8 (2.4 GHz) before the first real
            # matmul instead of running it at half clock.
            wt = warm_pool.tile([128, 512], dt.bfloat16)
            nc.gpsimd.memset(wt[:], 0.0)
            wps = ps1_pool.tile([128, 512], dt.float32, tag="ps1")
            for _ in range(9):
                nc.tensor.matmul(wps[:], wt[:, :128], wt[:], start=True, stop=True)

            b1t = b1_pool.tile([128, HT], dt.float32)
            nc.sync.dma_start(b1t[:], b1_d.ap())

            def load_w1(ht):
                w1t = w1_pool.tile([128, KT * 128], io_dt, name="w1t", tag="w1t")
                nc.sync.dma_start(w1t[:], w1_d.ap()[:, ht, :])
                return w1t

            w1_tiles = {0: load_w1(0)}

            # xt kt-major, matching the first h-tile's matmul order; the
            # second half rides the otherwise-idle SWDGE queue in parallel
            # so the whole tensor is resident before the first h-tile's
            # matmuls reach it
            xt = xt_pool.tile([128, KT, C], io_dt)
            for kt in range(KT // 2):
                for c0, n in chunks:
                    nc.sync.dma_start(
                        xt[:, kt, c0 : c0 + n], xt_v[:, kt, c0 : c0 + n]
                    )
                if kt == 0:
                    w1_tiles[1] = load_w1(1)
            for kt in range(KT // 2, KT):
                nc.gpsimd.dma_start(xt[:, kt, :], xt_v[:, kt, :])

            w2t = w2_pool.tile([128, HT, 1024], io_dt)
            h1 = h1_pool.tile([128, HT, C], io_dt)

            # ---- phase 1: H1T[h, tok] = relu(W1^T @ XT + b1) ----
            for ht in range(HT):
                w1t = w1_tiles.pop(ht)
                if ht + 2 < HT:
                    w1_tiles[ht + 2] = load_w1(ht + 2)
                # w2 spread piece-by-piece behind the w1 stream; all of it
                # is resident well before phase 2 needs it
                nc.sync.dma_start(w2t[:, ht, :], w2_v[:, ht, :])
                pss = [
                    ps1_pool.tile([128, 512], dt.float32, tag="ps1", name=f"pss{ci}")
                    for ci in range(len(chunks))
                ]
                # kt outer, chunk inner: each stationary w1 tile streams
                # all C columns (one LDWEIGHTS per len(chunks) matmuls)
                for kt in range(KT):
                    for ci, (c0, n) in enumerate(chunks):
                        nc.tensor.matmul(
                            pss[ci][:, :n],
                            w1t[:, kt * 128 : (kt + 1) * 128],
                            xt[:, kt, c0 : c0 + n],
                            start=(kt == 0),
                            stop=(kt == KT - 1),
                        )
                for ci, (c0, n) in enumerate(chunks):
                    nc.scalar.activation(
                        h1[:, ht, c0 : c0 + n],
                        pss[ci][:, :n],
                        mybir.ActivationFunctionType.Relu,
                        bias=b1t[:, ht : ht + 1],
                    )

            # ---- phase 2: Y[tok, d] = H1T^T @ W2 ----
            for tt in range(TT):
                ps2 = [
                    ps2_pool.tile([128, 512], dt.float32, tag="ps2", name=f"ps2_{dc}")
                    for dc in range(2)
                ]
                for hh in range(HT):
                    for dc in range(2):
                        nc.tensor.matmul(
                            ps2[dc][:],
                            h1[:, hh, tt * 128 : (tt + 1) * 128],
                            w2t[:, hh, dc * 512 : (dc + 1) * 512],
                            start=(hh == 0),
                            stop=(hh == HT - 1),
                        )
                # the very last PSUM drain goes in half-size pieces so the
                # final store (which cannot start before the last matmul)
                # is as small as possible
                drains = [(0, 512), (512, 512)]
                if tt == TT - 1:
                    drains = [(0, 512), (512, 256), (768, 256)]
                for d0, dn in drains:
                    ys = yst_pool.tile([128, 512], dt.float32)
                    nc.vector.tensor_copy(
                        ys[:, :dn], ps2[d0 // 512][:, d0 % 512 : d0 % 512 + dn]
                    )
                    nc.sync.dma_start(y_v[:, tt, d0 : d0 + dn], ys[:, :dn])

    nc.compile()
    return nc


def _get_nc(C: int):
    if C not in _BUILD_CACHE:
        _BUILD_CACHE[C] = build_nc(C)
    return _BUILD_CACHE[C]


def _prep_w1(W1e: np.ndarray) -> np.ndarray:
    # [D, H] -> [128, HT, KT*128]: w1[p, ht, kt*128+j] = W1[kt*128+p, ht*128+j]
    return np.ascontiguousarray(
        W1e.astype(BF16).reshape(KT, 128, HT, 128).transpose(1, 2, 0, 3)
        .reshape(128, HT, KT * 128)
    )


def kernel(x, W1, b1, W2, b2, assign, k, _want_trace=False):
    x = np.asarray(x, dtype=np.float32)
    W1 = np.asarray(W1, dtype=np.float32)
    b1 = np.asarray(b1, dtype=np.float32)
    W2 = np.asarray(W2, dtype=np.float32)
    b2 = np.asarray(b2, dtype=np.float32)
    assign = np.asarray(assign)
    kk = int(k)

    assert W1.shape[0] == E and W2.shape[0] == E, "expert count must be 8"
    Bx, Sx, Dx = x.shape
    T = Bx * Sx
    xf = x.reshape(T, Dx)
    xT = np.ascontiguousarray(xf.T.astype(BF16))  # [D, T] bf16
    a2 = assign.reshape(T, -1)

    idx = [np.nonzero((a2 == e).any(axis=1))[0] for e in range(E)]
    max_n = max(len(i) for i in idx)

    # capacity per device pass (multiple of 128); single pass for the
    # expected distribution, multiple passes if pathologically skewed
    C = min(max(1024, math.ceil(max_n / 128) * 128), 1280)
    n_pass = math.ceil(max(max_n, 1) / C)

    nc = _get_nc(C)

    w1_io = [_prep_w1(W1[e]) for e in range(E)]
    w2_io = [np.ascontiguousarray(W2[e].astype(BF16)) for e in range(E)]
    b1_io = [np.ascontiguousarray(b1[e].reshape(HT, 128).T) for e in range(E)]

    out_f = np.zeros((T, Dx), dtype=np.float32)
    trace_info = None

    for p in range(n_pass):
        in_maps = []
        for e in range(E):
            sl = idx[e][p * C : (p + 1) * C]
            xt_buf = np.zeros((Dx, C), dtype=BF16)
            if len(sl):
                xt_buf[:, : len(sl)] = xT[:, sl]
            in_maps.append(
                {
                    "xt": xt_buf,
                    "w1": w1_io[e],
                    "b1": b1_io[e],
                    "w2": w2_io[e],
                }
            )
        res = run_bass_kernel_spmd(
            nc,
            in_maps,
            core_ids=list(range(NCORES)),
            trace=_want_trace,
            trace_cores=list(range(NCORES)) if _want_trace else None,
        )
        if _want_trace:
            trace_info = res
        for e in range(E):
            sl = idx[e][p * C : (p + 1) * C]
            if len(sl):
                out_f[sl] += res.results[e]["y"][: len(sl)] + b2[e][None, :]

    out = (out_f * np.float32(1.0 / kk)).reshape(Bx, Sx, Dx)
    if _want_trace:
        return out, trace_info
    return out


# revision 13
# speedup vs baseline: 1.0137x; 1.0137x over previous
"""MoE layer (hash-routed, top-k=2, E=8 experts) on 8 Trainium2 NeuronCores.

Strategy: expert-parallel. Core e holds expert e's weights (W1[e], W2[e]).
The host routes: for each expert, gather the distinct tokens assigned to it
(assign[b,s,:] contains expert ids; a token contributes once per distinct
expert), transpose the gathered activations to [D, C] so the device never
has to transpose, run a dense 2-layer MLP per core, then scatter-add the
per-expert outputs back and divide by k.

Device kernel (per core), C = token capacity (padded, multiple of 128),
two phases, matmul operands in bf16 (fp32 PSUM accumulation):

  phase 1: for each of 32 h-tiles:
      H1T[h,tok] = relu(W1[ht]^T @ XT + b1)   (PSUM-accum over 8 d-tiles;
                                               each stationary w1 tile
                                               streams all C tokens, so one
                                               LDWEIGHTS serves 2 matmuls)
      full H1T [4096 x C] stays in SBUF (bf16)
  phase 2: for each 128-token tile:
      Y[tok,d] = H1T^T @ W2                   (PSUM pair accumulates over
                                               all 32 h-tiles, then copy
                                               to SBUF and DMA out)

All DMA rides the sync HWDGE ring (the SWDGE path costs ~2us per op and
fine-grained patterns make it far slower still).  Host pre-lays-out b1 as
[128, HT] and W1 as [128, HT, KT*128] so every transfer moves >=1KB
contiguous runs per partition (descriptor-lean; the original transposed
b1 pattern emitted 4096 4-byte descriptors and alone stalled the ring
for ~23us).  Emission follows exact consumption order: b1, w1[0], xt
(kt-major), w1[1], then per h-tile {w1 two ahead, w2 piece}, then y
stores.  A short PE warm-up bridges the ~7us NEFF preamble until the
first pieces land so real matmuls start at full clock (HAM 8/8).
"""

import math
import numpy as np
import ml_dtypes

import concourse.bass as bass
import concourse.mybir as mybir
import concourse.tile as tile
from concourse import bacc
from concourse.bass_utils import run_bass_kernel_spmd

dt = mybir.dt

B, S, D, H, E, NCORES = 4, 1024, 1024, 4096, 8, 8
KT = D // 128                  # 8 contraction tiles (d)
HT = H // 128                  # 32 h tiles
BF16 = np.dtype(ml_dtypes.bfloat16)

_BUILD_CACHE: dict = {}


def build_nc(C: int):
    """Build + compile the per-core Bass program for token capacity C."""
    assert C % 128 == 0
    TT = C // 128
    io_dt = dt.bfloat16

    nc = bacc.Bacc(
        "TRN2",
        target_bir_lowering=False,
        debug=False,
        num_devices=NCORES,
    )

    # host-side layouts (see kernel()): w1 [128, HT, KT*128] with
    # w1[p, ht, kt*128+j] = W1[kt*128+p, ht*128+j]; b1 [128, HT] with
    # b1[p, ht] = b1_vec[ht*128+p]
    xt_d = nc.dram_tensor("xt", [D, C], io_dt, kind="ExternalInput")
    w1_d = nc.dram_tensor("w1", [128, HT, KT * 128], io_dt, kind="ExternalInput")
    b1_d = nc.dram_tensor("b1", [128, HT], dt.float32, kind="ExternalInput")
    w2_d = nc.dram_tensor("w2", [H, D], io_dt, kind="ExternalInput")
    y_d = nc.dram_tensor("y", [C, D], dt.float32, kind="ExternalOutput")

    xt_v = xt_d.ap().rearrange("(kt p) c -> p kt c", p=128)
    w2_v = w2_d.ap().rearrange("(hh p) d -> p hh d", p=128)
    y_v = y_d.ap().rearrange("(tt p) d -> p tt d", p=128)

    chunks = [(c0, min(512, C - c0)) for c0 in range(0, C, 512)]

    # SBUF per-partition budget (bytes)
    need = (
        KT * C * 2            # xt
        + HT * C * 2          # h1
        + HT * 1024 * 2       # w2 (all of it)
        + 4 * KT * 128 * 2    # w1 stream
        + 3 * 512 * 4         # y stage
        + HT * 4              # b1
        + 512 * 2             # warm tile
    )
    assert need <= 190 * 1024, f"SBUF over budget: {need // 1024}KB for C={C}"

    with tile.TileContext(nc) as tc:
        with (
            tc.tile_pool(name="xt", bufs=1) as xt_pool,
            tc.tile_pool(name="b1", bufs=1) as b1_pool,
            tc.tile_pool(name="w1s", bufs=4) as w1_pool,
            tc.tile_pool(name="w2a", bufs=1) as w2_pool,
            tc.tile_pool(name="h1", bufs=1) as h1_pool,
            tc.tile_pool(name="yst", bufs=3) as yst_pool,
            tc.tile_pool(name="warm", bufs=1) as warm_pool,
            tc.tile_pool(name="ps1", bufs=4, space="PSUM") as ps1_pool,
            tc.tile_pool(name="ps2", bufs=4, space="PSUM") as ps2_pool,
        ):
            # PE warm-up: dependency-free bf16 matmuls bridge the NEFF
            # preamble until the first xt/w1 pieces land (~10us), so the
            # HAM clock gate reaches 8/8 (2.4 GHz) before the first real
            # matmul instead of running it at half clock.
            wt = warm_pool.tile([128, 512], dt.bfloat16)
            nc.gpsimd.memset(wt[:], 0.0)
            wps = ps1_pool.tile([128, 512], dt.float32, tag="ps1")
            for _ in range(9):
                nc.tensor.matmul(wps[:], wt[:, :128], wt[:], start=True, stop=True)

            b1t = b1_pool.tile([128, HT], dt.float32)
            nc.sync.dma_start(b1t[:], b1_d.ap())

            def load_w1(ht):
                w1t = w1_pool.tile([128, KT * 128], io_dt, name="w1t", tag="w1t")
                nc.sync.dma_start(w1t[:], w1_d.ap()[:, ht, :])
                return w1t

            w1_tiles = {0: load_w1(0)}

            # xt in three fat pieces, kt-major (first h-tile's matmul
            # order): every dma_start costs ~600ns of sync-engine issue
            # time, so fewer+fatter transfers fill SBUF sooner
            xt = xt_pool.tile([128, KT, C], io_dt)
            nc.sync.dma_start(xt[:, 0:2, :], xt_v[:, 0:2, :])
            w1_tiles[1] = load_w1(1)
            nc.sync.dma_start(xt[:, 2:5, :], xt_v[:, 2:5, :])
            nc.sync.dma_start(xt[:, 5:KT, :], xt_v[:, 5:KT, :])
            w1_tiles[2] = load_w1(2)

            w2t = w2_pool.tile([128, HT, 1024], io_dt)
            h1 = h1_pool.tile([128, HT, C], io_dt)

            # ---- phase 1: H1T[h, tok] = relu(W1^T @ XT + b1) ----
            for ht in range(HT):
                w1t = w1_tiles.pop(ht)
                if ht + 3 < HT:
                    w1_tiles[ht + 3] = load_w1(ht + 3)
                # w2 in fat 4-h-tile pieces spread behind the w1 stream
                # (FIFO ring: w1 prefetch depth 3 rides out each ~1MB w2
                # transfer); all resident well before phase 2 needs it
                if ht % 4 == 0:
                    nc.sync.dma_start(
                        w2t[:, ht : ht + 4, :], w2_v[:, ht : ht + 4, :]
                    )
                pss = [
                    ps1_pool.tile([128, 512], dt.float32, tag="ps1", name=f"pss{ci}")
                    for ci in range(len(chunks))
                ]
                # kt outer, chunk inner: each stationary w1 tile streams
                # all C columns (one LDWEIGHTS per len(chunks) matmuls)
                for kt in range(KT):
                    for ci, (c0, n) in enumerate(chunks):
                        nc.tensor.matmul(
                            pss[ci][:, :n],
                            w1t[:, kt * 128 : (kt + 1) * 128],
                            xt[:, kt, c0 : c0 + n],
                            start=(kt == 0),
                            stop=(kt == KT - 1),
                        )
                for ci, (c0, n) in enumerate(chunks):
                    nc.scalar.activation(
                        h1[:, ht, c0 : c0 + n],
                        pss[ci][:, :n],
                        mybir.ActivationFunctionType.Relu,
                        bias=b1t[:, ht : ht + 1],
                    )

            # ---- phase 2: Y[tok, d] = H1T^T @ W2 ----
            for tt in range(TT):
                ps2 = [
                    ps2_pool.tile([128, 512], dt.float32, tag="ps2", name=f"ps2_{dc}")
                    for dc in range(2)
                ]
                for hh in range(HT):
                    for dc in range(2):
                        nc.tensor.matmul(
                            ps2[dc][:],
                            h1[:, hh, tt * 128 : (tt + 1) * 128],
                            w2t[:, hh, dc * 512 : (dc + 1) * 512],
                            start=(hh == 0),
                            stop=(hh == HT - 1),
                        )
                # the very last PSUM drain goes in half-size pieces so the
                # final store (which cannot start before the last matmul)
                # is as small as possible
                drains = [(0, 512), (512, 512)]
                if tt == TT - 1:
                    drains = [(0, 512), (512, 256), (768, 256)]
                for d0, dn in drains:
                    ys = yst_pool.tile([128, 512], dt.float32)
                    nc.vector.tensor_copy(
                        ys[:, :dn], ps2[d0 // 512][:, d0 % 512 : d0 % 512 + dn]
                    )
                    nc.sync.dma_start(y_v[:, tt, d0 : d0 + dn], ys[:, :dn])

    nc.compile()
    return nc


def _get_nc(C: int):
    if C not in _BUILD_CACHE:
        _BUILD_CACHE[C] = build_nc(C)
    return _BUILD_CACHE[C]


def _prep_w1(W1e: np.ndarray) -> np.ndarray:
    # [D, H] -> [128, HT, KT*128]: w1[p, ht, kt*128+j] = W1[kt*128+p, ht*128+j]
    return np.ascontiguousarray(
        W1e.astype(BF16).reshape(KT, 128, HT, 128).transpose(1, 2, 0, 3)
        .reshape(128, HT, KT * 128)
    )


def kernel(x, W1, b1, W2, b2, assign, k, _want_trace=False):
    x = np.asarray(x, dtype=np.float32)
    W1 = np.asarray(W1, dtype=np.float32)
    b1 = np.asarray(b1, dtype=np.float32)
    W2 = np.asarray(W2, dtype=np.float32)
    b2 = np.asarray(b2, dtype=np.float32)
    assign = np.asarray(assign)
    kk = int(k)

    assert W1.shape[0] == E and W2.shape[0] == E, "expert count must be 8"
    Bx, Sx, Dx = x.shape
    T = Bx * Sx
    xf = x.reshape(T, Dx)
    xT = np.ascontiguousarray(xf.T.astype(BF16))  # [D, T] bf16
    a2 = assign.reshape(T, -1)

    idx = [np.nonzero((a2 == e).any(axis=1))[0] for e in range(E)]
    max_n = max(len(i) for i in idx)

    # capacity per device pass (multiple of 128); single pass for the
    # expected distribution, multiple passes if pathologically skewed
    C = min(max(1024, math.ceil(max_n / 128) * 128), 1280)
    n_pass = math.ceil(max(max_n, 1) / C)

    nc = _get_nc(C)

    w1_io = [_prep_w1(W1[e]) for e in range(E)]
    w2_io = [np.ascontiguousarray(W2[e].astype(BF16)) for e in range(E)]
    b1_io = [np.ascontiguousarray(b1[e].reshape(HT, 128).T) for e in range(E)]

    out_f = np.zeros((T, Dx), dtype=np.float32)
    trace_info = None

    for p in range(n_pass):
        in_maps = []
        for e in range(E):
            sl = idx[e][p * C : (p + 1) * C]
            xt_buf = np.zeros((Dx, C), dtype=BF16)
            if len(sl):
                xt_buf[:, : len(sl)] = xT[:, sl]
            in_maps.append(
                {
                    "xt": xt_buf,
                    "w1": w1_io[e],
                    "b1": b1_io[e],
                    "w2": w2_io[e],
                }
            )
        res = run_bass_kernel_spmd(
            nc,
            in_maps,
            core_ids=list(range(NCORES)),
            trace=_want_trace,
            trace_cores=list(range(NCORES)) if _want_trace else None,
        )
        if _want_trace:
            trace_info = res
        for e in range(E):
            sl = idx[e][p * C : (p + 1) * C]
            if len(sl):
                out_f[sl] += res.results[e]["y"][: len(sl)] + b2[e][None, :]

    out = (out_f * np.float32(1.0 / kk)).reshape(Bx, Sx, Dx)
    if _want_trace:
        return out, trace_info
    return out


# revision 14
# speedup vs baseline: 1.0156x; 1.0019x over previous
"""MoE layer (hash-routed, top-k=2, E=8 experts) on 8 Trainium2 NeuronCores.

Strategy: expert-parallel. Core e holds expert e's weights (W1[e], W2[e]).
The host routes: for each expert, gather the distinct tokens assigned to it
(assign[b,s,:] contains expert ids; a token contributes once per distinct
expert), transpose the gathered activations to [D, C] so the device never
has to transpose, run a dense 2-layer MLP per core, then scatter-add the
per-expert outputs back and divide by k.

Device kernel (per core), C = token capacity (padded, multiple of 128),
two phases, matmul operands in bf16 (fp32 PSUM accumulation):

  phase 1: for each of 32 h-tiles:
      H1T[h,tok] = relu(W1[ht]^T @ XT + b1)   (PSUM-accum over 8 d-tiles;
                                               each stationary w1 tile
                                               streams all C tokens, so one
                                               LDWEIGHTS serves 2 matmuls)
      full H1T [4096 x C] stays in SBUF (bf16)
  phase 2: for each 128-token tile:
      Y[tok,d] = H1T^T @ W2                   (PSUM pair accumulates over
                                               all 32 h-tiles, then copy
                                               to SBUF and DMA out)

All DMA rides the sync HWDGE ring (the SWDGE path costs ~2us per op and
fine-grained patterns make it far slower still).  Host pre-lays-out b1 as
[128, HT] and W1 as [128, HT, KT*128] so every transfer moves >=1KB
contiguous runs per partition (descriptor-lean; the original transposed
b1 pattern emitted 4096 4-byte descriptors and alone stalled the ring
for ~23us).  Emission follows exact consumption order: b1, w1[0], xt
(kt-major), w1[1], then per h-tile {w1 two ahead, w2 piece}, then y
stores.  A short PE warm-up bridges the ~7us NEFF preamble until the
first pieces land so real matmuls start at full clock (HAM 8/8).
"""

import math
import numpy as np
import ml_dtypes

import concourse.bass as bass
import concourse.mybir as mybir
import concourse.tile as tile
from concourse import bacc
from concourse.bass_utils import run_bass_kernel_spmd

dt = mybir.dt

B, S, D, H, E, NCORES = 4, 1024, 1024, 4096, 8, 8
KT = D // 128                  # 8 contraction tiles (d)
HT = H // 128                  # 32 h tiles
BF16 = np.dtype(ml_dtypes.bfloat16)

_BUILD_CACHE: dict = {}


def build_nc(C: int):
    """Build + compile the per-core Bass program for token capacity C."""
    assert C % 128 == 0
    TT = C // 128
    io_dt = dt.bfloat16

    nc = bacc.Bacc(
        "TRN2",
        target_bir_lowering=False,
        debug=False,
        num_devices=NCORES,
    )

    # host-side layouts (see kernel()): w1 [128, HT, KT*128] with
    # w1[p, ht, kt*128+j] = W1[kt*128+p, ht*128+j]; b1 [128, HT] with
    # b1[p, ht] = b1_vec[ht*128+p]
    xt_d = nc.dram_tensor("xt", [D, C], io_dt, kind="ExternalInput")
    w1_d = nc.dram_tensor("w1", [128, HT, KT * 128], io_dt, kind="ExternalInput")
    b1_d = nc.dram_tensor("b1", [128, HT], dt.float32, kind="ExternalInput")
    w2_d = nc.dram_tensor("w2", [H, D], io_dt, kind="ExternalInput")
    y_d = nc.dram_tensor("y", [C, D], dt.float32, kind="ExternalOutput")

    xt_v = xt_d.ap().rearrange("(kt p) c -> p kt c", p=128)
    w2_v = w2_d.ap().rearrange("(hh p) d -> p hh d", p=128)
    y_v = y_d.ap().rearrange("(tt p) d -> p tt d", p=128)

    chunks = [(c0, min(512, C - c0)) for c0 in range(0, C, 512)]

    # SBUF per-partition budget (bytes)
    need = (
        KT * C * 2            # xt
        + HT * C * 2          # h1
        + HT * 1024 * 2       # w2 (all of it)
        + 4 * KT * 128 * 2    # w1 stream
        + 3 * 512 * 4         # y stage
        + HT * 4              # b1
        + 512 * 2             # warm tile
    )
    assert need <= 190 * 1024, f"SBUF over budget: {need // 1024}KB for C={C}"

    with tile.TileContext(nc) as tc:
        with (
            tc.tile_pool(name="xt", bufs=1) as xt_pool,
            tc.tile_pool(name="b1", bufs=1) as b1_pool,
            tc.tile_pool(name="w1s", bufs=4) as w1_pool,
            tc.tile_pool(name="w2a", bufs=1) as w2_pool,
            tc.tile_pool(name="h1", bufs=1) as h1_pool,
            tc.tile_pool(name="yst", bufs=3) as yst_pool,
            tc.tile_pool(name="warm", bufs=1) as warm_pool,
            tc.tile_pool(name="ps1", bufs=4, space="PSUM") as ps1_pool,
            tc.tile_pool(name="ps2", bufs=4, space="PSUM") as ps2_pool,
        ):
            # PE warm-up: dependency-free bf16 matmuls bridge the NEFF
            # preamble until the first xt/w1 pieces land (~10us), so the
            # HAM clock gate reaches 8/8 (2.4 GHz) before the first real
            # matmul instead of running it at half clock.
            wt = warm_pool.tile([128, 512], dt.bfloat16)
            nc.gpsimd.memset(wt[:], 0.0)
            wps = ps1_pool.tile([128, 512], dt.float32, tag="ps1")
            for _ in range(13):
                nc.tensor.matmul(wps[:], wt[:, :128], wt[:], start=True, stop=True)

            b1t = b1_pool.tile([128, HT], dt.float32)

            def load_w1(ht):
                w1t = w1_pool.tile([128, KT * 128], io_dt, name="w1t", tag="w1t")
                nc.sync.dma_start(w1t[:], w1_d.ap()[:, ht, :])
                return w1t

            w1_tiles = {0: load_w1(0)}

            # xt in three fat pieces, kt-major (first h-tile's matmul
            # order): every dma_start costs ~600ns of sync-engine issue
            # time, so fewer+fatter transfers fill SBUF sooner
            xt = xt_pool.tile([128, KT, C], io_dt)
            nc.sync.dma_start(xt[:, 0:2, :], xt_v[:, 0:2, :])
            w1_tiles[1] = load_w1(1)
            nc.sync.dma_start(xt[:, 2:5, :], xt_v[:, 2:5, :])
            nc.sync.dma_start(xt[:, 5:KT, :], xt_v[:, 5:KT, :])
            # b1 is only needed by the first activation (not the first
            # matmul), so it queues behind xt on the FIFO ring
            nc.sync.dma_start(b1t[:], b1_d.ap())
            w1_tiles[2] = load_w1(2)

            w2t = w2_pool.tile([128, HT, 1024], io_dt)
            h1 = h1_pool.tile([128, HT, C], io_dt)

            # ---- phase 1: H1T[h, tok] = relu(W1^T @ XT + b1) ----
            for ht in range(HT):
                w1t = w1_tiles.pop(ht)
                if ht + 3 < HT:
                    w1_tiles[ht + 3] = load_w1(ht + 3)
                # w2 in fat 4-h-tile pieces spread behind the w1 stream
                # (FIFO ring: w1 prefetch depth 3 rides out each ~1MB w2
                # transfer); all resident well before phase 2 needs it
                if ht % 4 == 0:
                    nc.sync.dma_start(
                        w2t[:, ht : ht + 4, :], w2_v[:, ht : ht + 4, :]
                    )
                pss = [
                    ps1_pool.tile([128, 512], dt.float32, tag="ps1", name=f"pss{ci}")
                    for ci in range(len(chunks))
                ]
                # kt outer, chunk inner: each stationary w1 tile streams
                # all C columns (one LDWEIGHTS per len(chunks) matmuls)
                for kt in range(KT):
                    for ci, (c0, n) in enumerate(chunks):
                        nc.tensor.matmul(
                            pss[ci][:, :n],
                            w1t[:, kt * 128 : (kt + 1) * 128],
                            xt[:, kt, c0 : c0 + n],
                            start=(kt == 0),
                            stop=(kt == KT - 1),
                        )
                for ci, (c0, n) in enumerate(chunks):
                    nc.scalar.activation(
                        h1[:, ht, c0 : c0 + n],
                        pss[ci][:, :n],
                        mybir.ActivationFunctionType.Relu,
                        bias=b1t[:, ht : ht + 1],
                    )

            # ---- phase 2: Y[tok, d] = H1T^T @ W2 ----
            for tt in range(TT):
                ps2 = [
                    ps2_pool.tile([128, 512], dt.float32, tag="ps2", name=f"ps2_{dc}")
                    for dc in range(2)
                ]
                for hh in range(HT):
                    for dc in range(2):
                        nc.tensor.matmul(
                            ps2[dc][:],
                            h1[:, hh, tt * 128 : (tt + 1) * 128],
                            w2t[:, hh, dc * 512 : (dc + 1) * 512],
                            start=(hh == 0),
                            stop=(hh == HT - 1),
                        )
                # the very last PSUM drain goes in half-size pieces so the
                # final store (which cannot start before the last matmul)
                # is as small as possible
                drains = [(0, 512), (512, 512)]
                if tt == TT - 1:
                    drains = [(0, 512), (512, 256), (768, 256)]
                for d0, dn in drains:
                    ys = yst_pool.tile([128, 512], dt.float32)
                    nc.vector.tensor_copy(
                        ys[:, :dn], ps2[d0 // 512][:, d0 % 512 : d0 % 512 + dn]
                    )
                    nc.sync.dma_start(y_v[:, tt, d0 : d0 + dn], ys[:, :dn])

    nc.compile()
    return nc


def _get_nc(C: int):
    if C not in _BUILD_CACHE:
        _BUILD_CACHE[C] = build_nc(C)
    return _BUILD_CACHE[C]


def _prep_w1(W1e: np.ndarray) -> np.ndarray:
    # [D, H] -> [128, HT, KT*128]: w1[p, ht, kt*128+j] = W1[kt*128+p, ht*128+j]
    return np.ascontiguousarray(
        W1e.astype(BF16).reshape(KT, 128, HT, 128).transpose(1, 2, 0, 3)
        .reshape(128, HT, KT * 128)
    )


def kernel(x, W1, b1, W2, b2, assign, k, _want_trace=False):
    x = np.asarray(x, dtype=np.float32)
    W1 = np.asarray(W1, dtype=np.float32)
    b1 = np.asarray(b1, dtype=np.float32)
    W2 = np.asarray(W2, dtype=np.float32)
    b2 = np.asarray(b2, dtype=np.float32)
    assign = np.asarray(assign)
    kk = int(k)

    assert W1.shape[0] == E and W2.shape[0] == E, "expert count must be 8"
    Bx, Sx, Dx = x.shape
    T = Bx * Sx
    xf = x.reshape(T, Dx)
    xT = np.ascontiguousarray(xf.T.astype(BF16))  # [D, T] bf16
    a2 = assign.reshape(T, -1)

    idx = [np.nonzero((a2 == e).any(axis=1))[0] for e in range(E)]
    max_n = max(len(i) for i in idx)

    # capacity per device pass (multiple of 128); single pass for the
    # expected distribution, multiple passes if pathologically skewed
    C = min(max(1024, math.ceil(max_n / 128) * 128), 1280)
    n_pass = math.ceil(max(max_n, 1) / C)

    nc = _get_nc(C)

    w1_io = [_prep_w1(W1[e]) for e in range(E)]
    w2_io = [np.ascontiguousarray(W2[e].astype(BF16)) for e in range(E)]
    b1_io = [np.ascontiguousarray(b1[e].reshape(HT, 128).T) for e in range(E)]

    out_f = np.zeros((T, Dx), dtype=np.float32)
    trace_info = None

    for p in range(n_pass):
        in_maps = []
        for e in range(E):
            sl = idx[e][p * C : (p + 1) * C]
            xt_buf = np.zeros((Dx, C), dtype=BF16)
            if len(sl):
                xt_buf[:, : len(sl)] = xT[:, sl]
            in_maps.append(
                {
                    "xt": xt_buf,
                    "w1": w1_io[e],
                    "b1": b1_io[e],
                    "w2": w2_io[e],
                }
            )
        res = run_bass_kernel_spmd(
            nc,
            in_maps,
            core_ids=list(range(NCORES)),
            trace=_want_trace,
            trace_cores=list(range(NCORES)) if _want_trace else None,
        )
        if _want_trace:
            trace_info = res
        for e in range(E):
            sl = idx[e][p * C : (p + 1) * C]
            if len(sl):
                out_f[sl] += res.results[e]["y"][: len(sl)] + b2[e][None, :]

    out = (out_f * np.float32(1.0 / kk)).reshape(Bx, Sx, Dx)
    if _want_trace:
        return out, trace_info
    return out


# revision 15
# speedup vs baseline: 1.0169x; 1.0013x over previous
"""MoE layer (hash-routed, top-k=2, E=8 experts) on 8 Trainium2 NeuronCores.

Strategy: expert-parallel. Core e holds expert e's weights (W1[e], W2[e]).
The host routes: for each expert, gather the distinct tokens assigned to it
(assign[b,s,:] contains expert ids; a token contributes once per distinct
expert), transpose the gathered activations to [D, C] so the device never
has to transpose, run a dense 2-layer MLP per core, then scatter-add the
per-expert outputs back and divide by k.

Device kernel (per core), C = token capacity (padded, multiple of 128),
two phases, matmul operands in bf16 (fp32 PSUM accumulation):

  phase 1: for each of 32 h-tiles:
      H1T[h,tok] = relu(W1[ht]^T @ XT + b1)   (PSUM-accum over 8 d-tiles;
                                               each stationary w1 tile
                                               streams all C tokens, so one
                                               LDWEIGHTS serves 2 matmuls)
      full H1T [4096 x C] stays in SBUF (bf16)
  phase 2: for each 128-token tile:
      Y[tok,d] = H1T^T @ W2                   (PSUM pair accumulates over
                                               all 32 h-tiles, then copy
                                               to SBUF and DMA out)

All DMA rides the sync HWDGE ring (the SWDGE path costs ~2us per op and
fine-grained patterns make it far slower still).  Host pre-lays-out b1 as
[128, HT] and W1 as [128, HT, KT*128] so every transfer moves >=1KB
contiguous runs per partition (descriptor-lean; the original transposed
b1 pattern emitted 4096 4-byte descriptors and alone stalled the ring
for ~23us).  Emission follows exact consumption order: b1, w1[0], xt
(kt-major), w1[1], then per h-tile {w1 two ahead, w2 piece}, then y
stores.  A short PE warm-up bridges the ~7us NEFF preamble until the
first pieces land so real matmuls start at full clock (HAM 8/8).
"""

import math
import numpy as np
import ml_dtypes

import concourse.bass as bass
import concourse.mybir as mybir
import concourse.tile as tile
from concourse import bacc
from concourse.bass_utils import run_bass_kernel_spmd

dt = mybir.dt

B, S, D, H, E, NCORES = 4, 1024, 1024, 4096, 8, 8
KT = D // 128                  # 8 contraction tiles (d)
HT = H // 128                  # 32 h tiles
BF16 = np.dtype(ml_dtypes.bfloat16)

_BUILD_CACHE: dict = {}


def build_nc(C: int):
    """Build + compile the per-core Bass program for token capacity C."""
    assert C % 128 == 0
    TT = C // 128
    io_dt = dt.bfloat16

    nc = bacc.Bacc(
        "TRN2",
        target_bir_lowering=False,
        debug=False,
        num_devices=NCORES,
    )

    # host-side layouts (see kernel()): w1 [128, HT, KT*128] with
    # w1[p, ht, kt*128+j] = W1[kt*128+p, ht*128+j]; b1 [128, HT] with
    # b1[p, ht] = b1_vec[ht*128+p]
    # xt/w1/w2/b1 arrive in partition-major layouts (see kernel()) so every
    # DMA moves multi-KB contiguous runs per partition: descriptor count,
    # not bytes, limits HWDGE throughput (~78ns/descriptor across 16 SDMA
    # engines; 2KB descriptors cap at ~210GB/s, 6-16KB reach ~400GB/s)
    xt_d = nc.dram_tensor("xt", [128, KT, C], io_dt, kind="ExternalInput")
    w1_d = nc.dram_tensor("w1", [128, HT, KT * 128], io_dt, kind="ExternalInput")
    b1_d = nc.dram_tensor("b1", [128, HT], dt.float32, kind="ExternalInput")
    w2_d = nc.dram_tensor("w2", [128, HT, 1024], io_dt, kind="ExternalInput")
    y_d = nc.dram_tensor("y", [C, D], dt.float32, kind="ExternalOutput")

    xt_v = xt_d.ap()
    w2_v = w2_d.ap()
    y_v = y_d.ap().rearrange("(tt p) d -> p tt d", p=128)

    chunks = [(c0, min(512, C - c0)) for c0 in range(0, C, 512)]

    # SBUF per-partition budget (bytes)
    need = (
        KT * C * 2            # xt
        + HT * C * 2          # h1
        + HT * 1024 * 2       # w2 (all of it)
        + 4 * KT * 128 * 2    # w1 stream
        + 3 * 512 * 4         # y stage
        + HT * 4              # b1
        + 512 * 2             # warm tile
    )
    assert need <= 190 * 1024, f"SBUF over budget: {need // 1024}KB for C={C}"

    with tile.TileContext(nc) as tc:
        with (
            tc.tile_pool(name="xt", bufs=1) as xt_pool,
            tc.tile_pool(name="b1", bufs=1) as b1_pool,
            tc.tile_pool(name="w1s", bufs=4) as w1_pool,
            tc.tile_pool(name="w2a", bufs=1) as w2_pool,
            tc.tile_pool(name="h1", bufs=1) as h1_pool,
            tc.tile_pool(name="yst", bufs=3) as yst_pool,
            tc.tile_pool(name="warm", bufs=1) as warm_pool,
            tc.tile_pool(name="ps1", bufs=4, space="PSUM") as ps1_pool,
            tc.tile_pool(name="ps2", bufs=4, space="PSUM") as ps2_pool,
        ):
            # PE warm-up: dependency-free bf16 matmuls bridge the NEFF
            # preamble until the first xt/w1 pieces land (~10us), so the
            # HAM clock gate reaches 8/8 (2.4 GHz) before the first real
            # matmul instead of running it at half clock.
            wt = warm_pool.tile([128, 512], dt.bfloat16)
            nc.gpsimd.memset(wt[:], 0.0)
            wps = ps1_pool.tile([128, 512], dt.float32, tag="ps1")
            for _ in range(10):
                nc.tensor.matmul(wps[:], wt[:, :128], wt[:], start=True, stop=True)

            b1t = b1_pool.tile([128, HT], dt.float32)

            def load_w1(ht):
                w1t = w1_pool.tile([128, KT * 128], io_dt, name="w1t", tag="w1t")
                nc.sync.dma_start(w1t[:], w1_d.ap()[:, ht, :])
                return w1t

            w1_tiles = {0: load_w1(0)}

            # xt in three fat pieces, kt-major (first h-tile's matmul
            # order): every dma_start costs ~600ns of sync-engine issue
            # time, so fewer+fatter transfers fill SBUF sooner
            xt = xt_pool.tile([128, KT, C], io_dt)
            nc.sync.dma_start(xt[:, 0:2, :], xt_v[:, 0:2, :])
            w1_tiles[1] = load_w1(1)
            nc.sync.dma_start(xt[:, 2:5, :], xt_v[:, 2:5, :])
            nc.sync.dma_start(xt[:, 5:KT, :], xt_v[:, 5:KT, :])
            # b1 is only needed by the first activation (not the first
            # matmul), so it queues behind xt on the FIFO ring
            nc.sync.dma_start(b1t[:], b1_d.ap())
            w1_tiles[2] = load_w1(2)

            w2t = w2_pool.tile([128, HT, 1024], io_dt)
            h1 = h1_pool.tile([128, HT, C], io_dt)

            # ---- phase 1: H1T[h, tok] = relu(W1^T @ XT + b1) ----
            for ht in range(HT):
                w1t = w1_tiles.pop(ht)
                if ht + 3 < HT:
                    w1_tiles[ht + 3] = load_w1(ht + 3)
                # w2 in fat 8-h-tile pieces spread behind the w1 stream,
                # starting at ht=2 so they never contend with the xt
                # prologue (FIFO ring: w1 prefetch depth 3 rides out each
                # ~2MB transfer); all resident well before phase 2
                if ht % 8 == 2:
                    hh0 = (ht // 8) * 8
                    nc.sync.dma_start(
                        w2t[:, hh0 : hh0 + 8, :], w2_v[:, hh0 : hh0 + 8, :]
                    )
                pss = [
                    ps1_pool.tile([128, 512], dt.float32, tag="ps1", name=f"pss{ci}")
                    for ci in range(len(chunks))
                ]
                # kt outer, chunk inner: each stationary w1 tile streams
                # all C columns (one LDWEIGHTS per len(chunks) matmuls)
                for kt in range(KT):
                    for ci, (c0, n) in enumerate(chunks):
                        nc.tensor.matmul(
                            pss[ci][:, :n],
                            w1t[:, kt * 128 : (kt + 1) * 128],
                            xt[:, kt, c0 : c0 + n],
                            start=(kt == 0),
                            stop=(kt == KT - 1),
                        )
                for ci, (c0, n) in enumerate(chunks):
                    nc.scalar.activation(
                        h1[:, ht, c0 : c0 + n],
                        pss[ci][:, :n],
                        mybir.ActivationFunctionType.Relu,
                        bias=b1t[:, ht : ht + 1],
                    )

            # ---- phase 2: Y[tok, d] = H1T^T @ W2 ----
            for tt in range(TT):
                ps2 = [
                    ps2_pool.tile([128, 512], dt.float32, tag="ps2", name=f"ps2_{dc}")
                    for dc in range(2)
                ]
                for hh in range(HT):
                    for dc in range(2):
                        nc.tensor.matmul(
                            ps2[dc][:],
                            h1[:, hh, tt * 128 : (tt + 1) * 128],
                            w2t[:, hh, dc * 512 : (dc + 1) * 512],
                            start=(hh == 0),
                            stop=(hh == HT - 1),
                        )
                # the very last PSUM drain goes in half-size pieces so the
                # final store (which cannot start before the last matmul)
                # is as small as possible
                drains = [(0, 512), (512, 512)]
                if tt == TT - 1:
                    drains = [(0, 512), (512, 256), (768, 256)]
                for d0, dn in drains:
                    ys = yst_pool.tile([128, 512], dt.float32)
                    nc.vector.tensor_copy(
                        ys[:, :dn], ps2[d0 // 512][:, d0 % 512 : d0 % 512 + dn]
                    )
                    nc.sync.dma_start(y_v[:, tt, d0 : d0 + dn], ys[:, :dn])

    nc.compile()
    return nc


def _get_nc(C: int):
    if C not in _BUILD_CACHE:
        _BUILD_CACHE[C] = build_nc(C)
    return _BUILD_CACHE[C]


def _prep_w1(W1e: np.ndarray) -> np.ndarray:
    # [D, H] -> [128, HT, KT*128]: w1[p, ht, kt*128+j] = W1[kt*128+p, ht*128+j]
    return np.ascontiguousarray(
        W1e.astype(BF16).reshape(KT, 128, HT, 128).transpose(1, 2, 0, 3)
        .reshape(128, HT, KT * 128)
    )


def kernel(x, W1, b1, W2, b2, assign, k, _want_trace=False):
    x = np.asarray(x, dtype=np.float32)
    W1 = np.asarray(W1, dtype=np.float32)
    b1 = np.asarray(b1, dtype=np.float32)
    W2 = np.asarray(W2, dtype=np.float32)
    b2 = np.asarray(b2, dtype=np.float32)
    assign = np.asarray(assign)
    kk = int(k)

    assert W1.shape[0] == E and W2.shape[0] == E, "expert count must be 8"
    Bx, Sx, Dx = x.shape
    T = Bx * Sx
    xf = x.reshape(T, Dx)
    xT = np.ascontiguousarray(xf.T.astype(BF16))  # [D, T] bf16
    a2 = assign.reshape(T, -1)

    idx = [np.nonzero((a2 == e).any(axis=1))[0] for e in range(E)]
    max_n = max(len(i) for i in idx)

    # capacity per device pass (multiple of 128); single pass for the
    # expected distribution, multiple passes if pathologically skewed
    C = min(max(1024, math.ceil(max_n / 128) * 128), 1280)
    n_pass = math.ceil(max(max_n, 1) / C)

    nc = _get_nc(C)

    w1_io = [_prep_w1(W1[e]) for e in range(E)]
    # [H, D] -> [128, HT, D]: w2[p, hh, :] = W2[hh*128+p, :]
    w2_io = [
        np.ascontiguousarray(
            W2[e].astype(BF16).reshape(HT, 128, Dx).transpose(1, 0, 2)
        )
        for e in range(E)
    ]
    b1_io = [np.ascontiguousarray(b1[e].reshape(HT, 128).T) for e in range(E)]

    out_f = np.zeros((T, Dx), dtype=np.float32)
    trace_info = None

    for p in range(n_pass):
        in_maps = []
        for e in range(E):
            sl = idx[e][p * C : (p + 1) * C]
            # [128, KT, C]: xt[p, kt, c] = x[token sl[c], kt*128+p]
            xt_buf = np.zeros((128, KT, C), dtype=BF16)
            if len(sl):
                xt_buf[:, :, : len(sl)] = (
                    xT[:, sl].reshape(KT, 128, len(sl)).transpose(1, 0, 2)
                )
            in_maps.append(
                {
                    "xt": xt_buf,
                    "w1": w1_io[e],
                    "b1": b1_io[e],
                    "w2": w2_io[e],
                }
            )
        res = run_bass_kernel_spmd(
            nc,
            in_maps,
            core_ids=list(range(NCORES)),
            trace=_want_trace,
            trace_cores=list(range(NCORES)) if _want_trace else None,
        )
        if _want_trace:
            trace_info = res
        for e in range(E):
            sl = idx[e][p * C : (p + 1) * C]
            if len(sl):
                out_f[sl] += res.results[e]["y"][: len(sl)] + b2[e][None, :]

    out = (out_f * np.float32(1.0 / kk)).reshape(Bx, Sx, Dx)
    if _want_trace:
        return out, trace_info
    return out


# revision 16
# speedup vs baseline: 1.0183x; 1.0013x over previous
"""MoE layer (hash-routed, top-k=2, E=8 experts) on 8 Trainium2 NeuronCores.

Strategy: expert-parallel. Core e holds expert e's weights (W1[e], W2[e]).
The host routes: for each expert, gather the distinct tokens assigned to it
(assign[b,s,:] contains expert ids; a token contributes once per distinct
expert), transpose the gathered activations to [D, C] so the device never
has to transpose, run a dense 2-layer MLP per core, then scatter-add the
per-expert outputs back and divide by k.

Device kernel (per core), C = token capacity (padded, multiple of 128),
two phases, matmul operands in bf16 (fp32 PSUM accumulation):

  phase 1: for each of 32 h-tiles:
      H1T[h,tok] = relu(W1[ht]^T @ XT + b1)   (PSUM-accum over 8 d-tiles;
                                               each stationary w1 tile
                                               streams all C tokens, so one
                                               LDWEIGHTS serves 2 matmuls)
      full H1T [4096 x C] stays in SBUF (bf16)
  phase 2: for each 128-token tile:
      Y[tok,d] = H1T^T @ W2                   (PSUM pair accumulates over
                                               all 32 h-tiles, then copy
                                               to SBUF and DMA out)

All DMA rides the sync HWDGE ring (the SWDGE path costs ~2us per op and
fine-grained patterns make it far slower still).  Host pre-lays-out b1 as
[128, HT] and W1 as [128, HT, KT*128] so every transfer moves >=1KB
contiguous runs per partition (descriptor-lean; the original transposed
b1 pattern emitted 4096 4-byte descriptors and alone stalled the ring
for ~23us).  Emission follows exact consumption order: b1, w1[0], xt
(kt-major), w1[1], then per h-tile {w1 two ahead, w2 piece}, then y
stores.  A short PE warm-up bridges the ~7us NEFF preamble until the
first pieces land so real matmuls start at full clock (HAM 8/8).
"""

import math
import numpy as np
import ml_dtypes

import concourse.bass as bass
import concourse.mybir as mybir
import concourse.tile as tile
from concourse import bacc
from concourse.bass_utils import run_bass_kernel_spmd

dt = mybir.dt

B, S, D, H, E, NCORES = 4, 1024, 1024, 4096, 8, 8
KT = D // 128                  # 8 contraction tiles (d)
HT = H // 128                  # 32 h tiles
BF16 = np.dtype(ml_dtypes.bfloat16)

_BUILD_CACHE: dict = {}


def build_nc(C: int):
    """Build + compile the per-core Bass program for token capacity C."""
    assert C % 128 == 0
    TT = C // 128
    io_dt = dt.bfloat16

    nc = bacc.Bacc(
        "TRN2",
        target_bir_lowering=False,
        debug=False,
        num_devices=NCORES,
    )

    # host-side layouts (see kernel()): w1 [128, HT, KT*128] with
    # w1[p, ht, kt*128+j] = W1[kt*128+p, ht*128+j]; b1 [128, HT] with
    # b1[p, ht] = b1_vec[ht*128+p]
    # xt/w1/w2/b1 arrive in partition-major layouts (see kernel()) so every
    # DMA moves multi-KB contiguous runs per partition: descriptor count,
    # not bytes, limits HWDGE throughput (~78ns/descriptor across 16 SDMA
    # engines; 2KB descriptors cap at ~210GB/s, 6-16KB reach ~400GB/s)
    xt_d = nc.dram_tensor("xt", [128, KT, C], io_dt, kind="ExternalInput")
    w1_d = nc.dram_tensor("w1", [128, HT, KT * 128], io_dt, kind="ExternalInput")
    b1_d = nc.dram_tensor("b1", [128, HT], dt.float32, kind="ExternalInput")
    w2_d = nc.dram_tensor("w2", [128, HT, 1024], io_dt, kind="ExternalInput")
    y_d = nc.dram_tensor("y", [C, D], dt.float32, kind="ExternalOutput")

    xt_v = xt_d.ap()
    w2_v = w2_d.ap()
    y_v = y_d.ap().rearrange("(tt p) d -> p tt d", p=128)

    chunks = [(c0, min(512, C - c0)) for c0 in range(0, C, 512)]

    # SBUF per-partition budget (bytes)
    need = (
        KT * C * 2            # xt
        + HT * C * 2          # h1
        + HT * 1024 * 2       # w2 (all of it)
        + 4 * KT * 128 * 2    # w1 stream
        + 3 * 512 * 4         # y stage
        + HT * 4              # b1
        + 512 * 2             # warm tile
    )
    assert need <= 190 * 1024, f"SBUF over budget: {need // 1024}KB for C={C}"

    with tile.TileContext(nc) as tc:
        with (
            tc.tile_pool(name="xt", bufs=1) as xt_pool,
            tc.tile_pool(name="b1", bufs=1) as b1_pool,
            tc.tile_pool(name="w1s", bufs=4) as w1_pool,
            tc.tile_pool(name="w2a", bufs=1) as w2_pool,
            tc.tile_pool(name="h1", bufs=1) as h1_pool,
            tc.tile_pool(name="yst", bufs=3) as yst_pool,
            tc.tile_pool(name="warm", bufs=1) as warm_pool,
            tc.tile_pool(name="ps1", bufs=4, space="PSUM") as ps1_pool,
            tc.tile_pool(name="ps2", bufs=4, space="PSUM") as ps2_pool,
        ):
            # PE warm-up: dependency-free bf16 matmuls bridge the NEFF
            # preamble until the first xt/w1 pieces land (~10us), so the
            # HAM clock gate reaches 8/8 (2.4 GHz) before the first real
            # matmul instead of running it at half clock.
            wt = warm_pool.tile([128, 512], dt.bfloat16)
            nc.gpsimd.memset(wt[:], 0.0)
            wps = ps1_pool.tile([128, 512], dt.float32, tag="ps1")
            for _ in range(12):
                nc.tensor.matmul(wps[:], wt[:, :128], wt[:], start=True, stop=True)

            b1t = b1_pool.tile([128, HT], dt.float32)

            def load_w1(ht):
                w1t = w1_pool.tile([128, KT * 128], io_dt, name="w1t", tag="w1t")
                nc.sync.dma_start(w1t[:], w1_d.ap()[:, ht, :])
                return w1t

            w1_tiles = {0: load_w1(0)}

            # xt in three fat pieces, kt-major (first h-tile's matmul
            # order): every dma_start costs ~600ns of sync-engine issue
            # time, so fewer+fatter transfers fill SBUF sooner
            xt = xt_pool.tile([128, KT, C], io_dt)
            nc.sync.dma_start(xt[:, 0:1, :], xt_v[:, 0:1, :])
            w1_tiles[1] = load_w1(1)
            nc.sync.dma_start(xt[:, 1:3, :], xt_v[:, 1:3, :])
            nc.sync.dma_start(xt[:, 3:6, :], xt_v[:, 3:6, :])
            nc.sync.dma_start(xt[:, 6:KT, :], xt_v[:, 6:KT, :])
            # b1 is only needed by the first activation (not the first
            # matmul), so it queues behind xt on the FIFO ring
            nc.sync.dma_start(b1t[:], b1_d.ap())
            w1_tiles[2] = load_w1(2)

            w2t = w2_pool.tile([128, HT, 1024], io_dt)
            h1 = h1_pool.tile([128, HT, C], io_dt)

            # ---- phase 1: H1T[h, tok] = relu(W1^T @ XT + b1) ----
            for ht in range(HT):
                w1t = w1_tiles.pop(ht)
                if ht + 3 < HT:
                    w1_tiles[ht + 3] = load_w1(ht + 3)
                # w2 in fat 8-h-tile pieces spread behind the w1 stream,
                # starting at ht=2 so they never contend with the xt
                # prologue (FIFO ring: w1 prefetch depth 3 rides out each
                # ~2MB transfer); all resident well before phase 2
                if ht % 8 == 2:
                    hh0 = (ht // 8) * 8
                    nc.sync.dma_start(
                        w2t[:, hh0 : hh0 + 8, :], w2_v[:, hh0 : hh0 + 8, :]
                    )
                pss = [
                    ps1_pool.tile([128, 512], dt.float32, tag="ps1", name=f"pss{ci}")
                    for ci in range(len(chunks))
                ]
                # kt outer, chunk inner: each stationary w1 tile streams
                # all C columns (one LDWEIGHTS per len(chunks) matmuls)
                for kt in range(KT):
                    for ci, (c0, n) in enumerate(chunks):
                        nc.tensor.matmul(
                            pss[ci][:, :n],
                            w1t[:, kt * 128 : (kt + 1) * 128],
                            xt[:, kt, c0 : c0 + n],
                            start=(kt == 0),
                            stop=(kt == KT - 1),
                        )
                for ci, (c0, n) in enumerate(chunks):
                    nc.scalar.activation(
                        h1[:, ht, c0 : c0 + n],
                        pss[ci][:, :n],
                        mybir.ActivationFunctionType.Relu,
                        bias=b1t[:, ht : ht + 1],
                    )

            # ---- phase 2: Y[tok, d] = H1T^T @ W2 ----
            # last token-tile handled separately: its dc=0 accumulation is
            # drained while dc=1 still computes, and dc=1 drains in two
            # half-width pieces, so the tail after the very last matmul is
            # one [128,256] copy + one 128KB store instead of a full 1KB
            # per-partition drain chain
            for tt in range(TT - 1):
                ps2 = [
                    ps2_pool.tile([128, 512], dt.float32, tag="ps2", name=f"ps2_{dc}")
                    for dc in range(2)
                ]
                for hh in range(HT):
                    for dc in range(2):
                        nc.tensor.matmul(
                            ps2[dc][:],
                            h1[:, hh, tt * 128 : (tt + 1) * 128],
                            w2t[:, hh, dc * 512 : (dc + 1) * 512],
                            start=(hh == 0),
                            stop=(hh == HT - 1),
                        )
                for dc in range(2):
                    ys = yst_pool.tile([128, 512], dt.float32)
                    nc.vector.tensor_copy(ys[:], ps2[dc][:])
                    nc.sync.dma_start(y_v[:, tt, dc * 512 : (dc + 1) * 512], ys[:])

            tt = TT - 1
            segs = [(0, 512), (512, 256), (768, 256)]
            psl = [
                ps2_pool.tile([128, 512], dt.float32, tag="ps2", name=f"psl{si}")
                for si in range(len(segs))
            ]
            for si, (d0, dn) in enumerate(segs):
                for hh in range(HT):
                    nc.tensor.matmul(
                        psl[si][:, :dn],
                        h1[:, hh, tt * 128 : (tt + 1) * 128],
                        w2t[:, hh, d0 : d0 + dn],
                        start=(hh == 0),
                        stop=(hh == HT - 1),
                    )
                ys = yst_pool.tile([128, 512], dt.float32)
                nc.vector.tensor_copy(ys[:, :dn], psl[si][:, :dn])
                nc.sync.dma_start(y_v[:, tt, d0 : d0 + dn], ys[:, :dn])

    nc.compile()
    return nc


def _get_nc(C: int):
    if C not in _BUILD_CACHE:
        _BUILD_CACHE[C] = build_nc(C)
    return _BUILD_CACHE[C]


def _prep_w1(W1e: np.ndarray) -> np.ndarray:
    # [D, H] -> [128, HT, KT*128]: w1[p, ht, kt*128+j] = W1[kt*128+p, ht*128+j]
    return np.ascontiguousarray(
        W1e.astype(BF16).reshape(KT, 128, HT, 128).transpose(1, 2, 0, 3)
        .reshape(128, HT, KT * 128)
    )


def kernel(x, W1, b1, W2, b2, assign, k, _want_trace=False):
    x = np.asarray(x, dtype=np.float32)
    W1 = np.asarray(W1, dtype=np.float32)
    b1 = np.asarray(b1, dtype=np.float32)
    W2 = np.asarray(W2, dtype=np.float32)
    b2 = np.asarray(b2, dtype=np.float32)
    assign = np.asarray(assign)
    kk = int(k)

    assert W1.shape[0] == E and W2.shape[0] == E, "expert count must be 8"
    Bx, Sx, Dx = x.shape
    T = Bx * Sx
    xf = x.reshape(T, Dx)
    xT = np.ascontiguousarray(xf.T.astype(BF16))  # [D, T] bf16
    a2 = assign.reshape(T, -1)

    idx = [np.nonzero((a2 == e).any(axis=1))[0] for e in range(E)]
    max_n = max(len(i) for i in idx)

    # capacity per device pass (multiple of 128); single pass for the
    # expected distribution, multiple passes if pathologically skewed
    C = min(max(1024, math.ceil(max_n / 128) * 128), 1280)
    n_pass = math.ceil(max(max_n, 1) / C)

    nc = _get_nc(C)

    w1_io = [_prep_w1(W1[e]) for e in range(E)]
    # [H, D] -> [128, HT, D]: w2[p, hh, :] = W2[hh*128+p, :]
    w2_io = [
        np.ascontiguousarray(
            W2[e].astype(BF16).reshape(HT, 128, Dx).transpose(1, 0, 2)
        )
        for e in range(E)
    ]
    b1_io = [np.ascontiguousarray(b1[e].reshape(HT, 128).T) for e in range(E)]

    out_f = np.zeros((T, Dx), dtype=np.float32)
    trace_info = None

    for p in range(n_pass):
        in_maps = []
        for e in range(E):
            sl = idx[e][p * C : (p + 1) * C]
            # [128, KT, C]: xt[p, kt, c] = x[token sl[c], kt*128+p]
            xt_buf = np.zeros((128, KT, C), dtype=BF16)
            if len(sl):
                xt_buf[:, :, : len(sl)] = (
                    xT[:, sl].reshape(KT, 128, len(sl)).transpose(1, 0, 2)
                )
            in_maps.append(
                {
                    "xt": xt_buf,
                    "w1": w1_io[e],
                    "b1": b1_io[e],
                    "w2": w2_io[e],
                }
            )
        res = run_bass_kernel_spmd(
            nc,
            in_maps,
            core_ids=list(range(NCORES)),
            trace=_want_trace,
            trace_cores=list(range(NCORES)) if _want_trace else None,
        )
        if _want_trace:
            trace_info = res
        for e in range(E):
            sl = idx[e][p * C : (p + 1) * C]
            if len(sl):
                out_f[sl] += res.results[e]["y"][: len(sl)] + b2[e][None, :]

    out = (out_f * np.float32(1.0 / kk)).reshape(Bx, Sx, Dx)
    if _want_trace:
        return out, trace_info
    return out
